# revision 11
# baseline (speedup 1.0000x reference)
"""Trainium2 Bass kernel for additive (Bahdanau-style) attention.

reference:
    energy = tanh(features @ Wf + hidden @ Wh + b_att)   # [B, L, H]
    scores = energy @ v                                   # [B, L]
    attn   = softmax(scores, axis=1)                      # [B, L]
    context = einsum('blf,bl->bf', features, attn)        # [B, F]

Sharding: data-parallel over batch B=64 across 8 cores (8 batches/core),
small weights replicated.

Per-core design (per batch b, per l-group of 512 rows):
  - DMA features l-group natively [l=128, f] with fp32->bf16 cast (SWDGE)
  - PE-transpose (bf16) into [f, l] tiles; DVE evacuates PSUM->SBUF
  - main matmul projT[h, l] += Wf_chunk.T @ featT_chunk (bf16, fp32 PSUM)
  - ACT evacuates PSUM with fused tanh(proj + c[b][h]) (bias per-partition)
  - scores[l] = v . energy via thin PE matmul (accumulate over h-chunks)
  - softmax handled partition-major: exp on ACT, partition-sum via
    ones-matmul broadcast trick, normalize on DVE
  - context accumulated on PE with unnormalized exp weights, scaled by
    1/sum at the end of the batch
"""

import numpy as np

B_TOTAL = 64
B_LOCAL = 8
L = 2048
F = 512
H = 512
N_CORES = 8

_CACHE = {}


def _build(use_bf16=True):
    import concourse.bass as bass
    import concourse.tile as tile
    from concourse import bacc, mybir

    dt = mybir.dt
    feat_dt = dt.float16 if use_bf16 else dt.float32r
    AF = mybir.ActivationFunctionType

    LG = 512                 # l-group size
    n_lg = L // LG           # 4
    n_lt = LG // 128         # 4  (128-row subtiles per l-group)
    n_fc = F // 128          # 4  (feature chunks)
    n_hc = H // 128          # 4  (hidden chunks)
    n_col = n_lg * n_lt      # 16 (score columns per batch, partition-major)

    nc = bacc.Bacc("TRN2", target_bir_lowering=False, debug=False, num_devices=1)

    feats = nc.dram_tensor("features", [B_LOCAL, L, F], dt.float32, kind="ExternalInput").ap()
    hidden = nc.dram_tensor("hidden_state", [B_LOCAL, H], dt.float32, kind="ExternalInput").ap()
    w_att = nc.dram_tensor("W_att", [F + H, H], dt.float32, kind="ExternalInput").ap()
    b_att = nc.dram_tensor("b_att", [1, H], dt.float32, kind="ExternalInput").ap()
    v_in = nc.dram_tensor("v", [1, H], dt.float32, kind="ExternalInput").ap()
    ident_in = nc.dram_tensor("ident", [128, 128], dt.float32, kind="ExternalInput").ap()
    ones_in = nc.dram_tensor("ones", [128, 128], dt.float32, kind="ExternalInput").ap()

    ctx_out = nc.dram_tensor("context", [B_LOCAL, F], dt.float32, kind="ExternalOutput").ap()
    attn_out = nc.dram_tensor("attention_weights", [B_LOCAL, L], dt.float32, kind="ExternalOutput").ap()

    with tile.TileContext(nc) as tc:
        with tc.tile_pool(name="const", bufs=1) as const:
            identf = const.tile([128, 128], dt.float32, tag="identf")
            nc.sync.dma_start(identf[:], ident_in[:])
            ones_lp = const.tile([128, 128], feat_dt, tag="ones_lp")
            nc.gpsimd.dma_start(ones_lp[:], ones_in[:])  # cast f32->fp16
            if use_bf16:
                identb = const.tile([128, 128], feat_dt, tag="identb")
                nc.gpsimd.dma_start(identb[:], ident_in[:])  # cast f32->bf16
            else:
                identb = identf.bitcast(dt.float32r)
            # Wf chunks [f=128, H] in feat dtype
            wf = []
            for fc in range(n_fc):
                t = const.tile([128, H], feat_dt, tag=f"wf{fc}")
                if use_bf16:
                    nc.gpsimd.dma_start(t[:], w_att[fc * 128:(fc + 1) * 128, :])
                else:
                    nc.sync.dma_start(t.bitcast(dt.float32)[:], w_att[fc * 128:(fc + 1) * 128, :])
                wf.append(t)
            # persistent small tensors built in precompute below
            vt = const.tile([128, n_hc], feat_dt, tag="vt")          # v, h-chunk cols
            ct = const.tile([128, n_hc * B_LOCAL], dt.float32, tag="ct")  # c[b][h], per h-chunk

            # ---- precompute c = hidden @ Wh + b_att (fp32), vT ----
            with tc.tile_pool(name="pre", bufs=1) as pre, \
                 tc.tile_pool(name="prepsum", bufs=2, space="PSUM") as prepsum:
                hid = pre.tile([B_LOCAL, H], dt.float32, tag="hid")
                nc.sync.dma_start(hid[:], hidden[:])
                wh = []
                for fc in range(n_fc):
                    t = pre.tile([128, H], dt.float32, tag=f"wh{fc}")
                    nc.sync.dma_start(t[:], w_att[F + fc * 128:F + (fc + 1) * 128, :])
                    wh.append(t)
                brow = pre.tile([1, H], dt.float32, tag="brow")
                nc.sync.dma_start(brow[:], b_att[:])
                vrow = pre.tile([1, H], dt.float32, tag="vrow")
                nc.sync.dma_start(vrow[:], v_in[:])

                # hidden transposed: ht[fc] = [f=128, B_LOCAL]
                ht = []
                for fc in range(n_fc):
                    p = prepsum.tile([128, B_LOCAL], dt.float32, tag="pre_t")
                    nc.tensor.transpose(
                        p[:], hid[:, fc * 128:(fc + 1) * 128], identf[:B_LOCAL, :B_LOCAL])
                    t = pre.tile([128, B_LOCAL], dt.float32, tag=f"ht{fc}")
                    nc.vector.tensor_copy(t[:], p[:])
                    ht.append(t)
                # b_att transposed: battT [128, n_hc] (col hc = b_att[hc*128:...])
                pb = prepsum.tile([128, n_hc], dt.float32, tag="pre_t")
                for hc in range(n_hc):
                    nc.tensor.transpose(
                        pb[:, hc:hc + 1], brow[0:1, hc * 128:(hc + 1) * 128], identf[0:1, 0:1])
                battT = pre.tile([128, n_hc], dt.float32, tag="battT")
                nc.vector.tensor_copy(battT[:], pb[:])
                # v transposed -> vt (cast to feat dtype)
                pv = prepsum.tile([128, n_hc], dt.float32, tag="pre_t")
                for hc in range(n_hc):
                    nc.tensor.transpose(
                        pv[:, hc:hc + 1], vrow[0:1, hc * 128:(hc + 1) * 128], identf[0:1, 0:1])
                if use_bf16:
                    nc.vector.tensor_copy(vt[:], pv[:])
                else:
                    nc.vector.tensor_copy(vt.bitcast(dt.float32)[:], pv[:])
                # cT[h=128, b] per h-chunk: sum_f Wh[f, h] * hT[f, b] + b_att[h]
                for hc in range(n_hc):
                    p = prepsum.tile([128, B_LOCAL], dt.float32, tag="pre_t")
                    for fc in range(n_fc):
                        nc.tensor.matmul(
                            p[:], lhsT=wh[fc][:, hc * 128:(hc + 1) * 128], rhs=ht[fc][:],
                            start=(fc == 0), stop=(fc == n_fc - 1))
                    nc.vector.tensor_scalar_add(
                        ct[:, hc * B_LOCAL:(hc + 1) * B_LOCAL], p[:], battT[:, hc:hc + 1])

            # ---- main pipeline (software-pipelined over (b, lg) steps) ----
            # stage head(k):  DMA nf, PE transposes, main MMs, ACT tanh -> en
            # stage tailA(k-1): scores MMs (en ready long ago), DVE srow copy
            # stage tailB(k-2): spm transposes, ACT exp, DVE fp16 cast
            # stage tailC(k-3): ctx MMs (exp ready long ago)
            # epilogue(b): after ctx of last lg of b
            with tc.tile_pool(name="nf", bufs=6) as nf_pool, \
                 tc.tile_pool(name="ft", bufs=n_fc) as ft_pool, \
                 tc.tile_pool(name="en", bufs=2 * n_hc) as en_pool, \
                 tc.tile_pool(name="srow", bufs=3) as srow_pool, \
                 tc.tile_pool(name="bt", bufs=3) as bt_pool, \
                 tc.tile_pool(name="psT", bufs=2, space="PSUM") as psT_pool, \
                 tc.tile_pool(name="psP", bufs=4, space="PSUM") as psP_pool, \
                 tc.tile_pool(name="psS", bufs=1, space="PSUM") as psS_pool, \
                 tc.tile_pool(name="psC", bufs=1, space="PSUM") as psC_pool:

                steps = [(b, lg) for b in range(B_LOCAL) for lg in range(n_lg)]
                st = {}        # step index -> dict of live tiles
                bstate = {}    # batch -> dict (exp_pm, exp_bf, pctx)

                def head(k):
                    b, lg = steps[k]
                    if lg == 0:
                        bstate[b] = {
                            "exp_pm": bt_pool.tile([128, n_col], dt.float32, tag="exp_pm", name="exp_pm"),
                            "exp_bf": bt_pool.tile([128, n_col], feat_dt, tag="exp_bf", name="exp_bf"),
                        }
                    nf = nf_pool.tile([128, n_lt * F], feat_dt, tag="nf")
                    src = feats[b, lg * LG:(lg + 1) * LG, :].rearrange(
                        "(n p) f -> p n f", p=128)
                    nc.gpsimd.dma_start(nf[:].rearrange("p (n f) -> p n f", f=F), src)
                    ft2 = []
                    for fp in range(n_fc // 2):
                        pT = psT_pool.tile([128, 2 * LG], feat_dt, tag="pT")
                        for half in range(2):
                            fc = 2 * fp + half
                            for lt in range(n_lt):
                                nc.tensor.transpose(
                                    pT[:, half * LG + lt * 128: half * LG + (lt + 1) * 128],
                                    nf[:, lt * F + fc * 128: lt * F + (fc + 1) * 128],
                                    identb[:])
                        t = ft_pool.tile([128, 2 * LG], feat_dt, tag="ft")
                        nc.vector.tensor_copy(t[:], pT[:])
                        ft2.append(t)
                    ft = [ft2[fc // 2][:, (fc % 2) * LG:(fc % 2 + 1) * LG] for fc in range(n_fc)]
                    st[k] = {"nf": nf, "ft": ft}

                def head2(k):
                    b, lg = steps[k]
                    s = st[k]
                    ft = s.pop("ft")
                    en = []
                    for hc in range(n_hc):
                        pproj = psP_pool.tile([128, LG], dt.float32, tag="pproj")
                        for fc in range(n_fc):
                            nc.tensor.matmul(
                                pproj[:],
                                lhsT=wf[fc][:, hc * 128:(hc + 1) * 128],
                                rhs=ft[fc][:],
                                start=(fc == 0), stop=(fc == n_fc - 1))
                        e = en_pool.tile([128, LG], feat_dt, tag="en")
                        nc.scalar.activation(
                            e[:], pproj[:], AF.Tanh,
                            bias=ct[:, hc * B_LOCAL + b: hc * B_LOCAL + b + 1])
                        en.append(e)
                    s["en"] = en

                def tailA(k):
                    s = st[k]
                    psc = psS_pool.tile([1, LG], dt.float32, tag="psSm")
                    for hc in range(n_hc):
                        nc.tensor.matmul(
                            psc[:], lhsT=vt[:, hc:hc + 1], rhs=s["en"][hc][:],
                            start=(hc == 0), stop=(hc == n_hc - 1))
                    srow = srow_pool.tile([1, LG], dt.float32, tag="srow")
                    nc.vector.tensor_copy(srow[:], psc[:])
                    s["srow"] = srow
                    del s["en"]

                def tailB(k):
                    b, lg = steps[k]
                    s = st[k]
                    bs = bstate[b]
                    pspm = psS_pool.tile([128, n_lt], dt.float32, tag="psSm")
                    for lt in range(n_lt):
                        nc.tensor.transpose(
                            pspm[:, lt:lt + 1],
                            s["srow"][0:1, lt * 128:(lt + 1) * 128],
                            identf[0:1, 0:1])
                    c0 = lg * n_lt
                    nc.scalar.activation(
                        bs["exp_pm"][:, c0:c0 + n_lt], pspm[:], AF.Exp)
                    nc.vector.tensor_copy(
                        bs["exp_bf"][:, c0:c0 + n_lt], bs["exp_pm"][:, c0:c0 + n_lt])
                    del s["srow"]

                def tailC(k):
                    b, lg = steps[k]
                    s = st[k]
                    bs = bstate[b]
                    if lg == 0:
                        bs["pctx"] = psC_pool.tile([1, F], dt.float32, tag="pctx", name="pctx")
                    c0 = lg * n_lt
                    for lt in range(n_lt):
                        nc.tensor.matmul(
                            bs["pctx"][:],
                            lhsT=bs["exp_bf"][:, c0 + lt:c0 + lt + 1],
                            rhs=s["nf"][:, lt * F:(lt + 1) * F],
                            start=(lg == 0 and lt == 0),
                            stop=(lg == n_lg - 1 and lt == n_lt - 1))
                    del st[k]

                def epilogue(b):
                    bs = bstate.pop(b)
                    pgs = psS_pool.tile([128, n_col], dt.float32, tag="psSm")
                    nc.tensor.matmul(
                        pgs[:], lhsT=ones_lp[:], rhs=bs["exp_bf"][:],
                        start=True, stop=True)
                    ssum = bt_pool.tile([128, 1], dt.float32, tag="ssum")
                    nc.vector.reduce_sum(ssum[:], pgs[:], axis=mybir.AxisListType.X)
                    rsum = bt_pool.tile([128, 1], dt.float32, tag="rsum")
                    nc.vector.reciprocal(rsum[:], ssum[:])
                    attn_pm = bt_pool.tile([128, n_col], dt.float32, tag="attn_pm")
                    nc.vector.tensor_scalar_mul(attn_pm[:], bs["exp_pm"][:], rsum[:, 0:1])
                    pat = psT_pool.tile([n_col, 128], dt.float32, tag="pT")
                    nc.tensor.transpose(pat[:], attn_pm[:], identf[:])
                    attn_rm = bt_pool.tile([n_col, 128], dt.float32, tag="attn_rm")
                    nc.vector.tensor_copy(attn_rm[:], pat[:])
                    nc.sync.dma_start(
                        attn_out[b].rearrange("(a c) -> a c", a=n_col), attn_rm[:])
                    ctx_sb = bt_pool.tile([1, F], dt.float32, tag="ctx_sb")
                    nc.vector.tensor_scalar_mul(ctx_sb[:], bs["pctx"][:], rsum[0:1, 0:1])
                    nc.sync.dma_start(ctx_out[b:b + 1, :], ctx_sb[:])

                n_steps = len(steps)
                for k in range(n_steps + 3):
                    if k < n_steps:
                        head(k)
                    if k >= 1 and k - 1 < n_steps:
                        tailA(k - 1)
                    if k >= 3 and k - 3 < n_steps:
                        tailC(k - 3)
                        if steps[k - 3][1] == n_lg - 1:
                            epilogue(steps[k - 3][0])
                    if k < n_steps:
                        head2(k)
                    if k >= 2 and k - 2 < n_steps:
                        tailB(k - 2)

    nc.compile()
    return nc


def _get_nc(use_bf16=True):
    key = ("bf16" if use_bf16 else "f32r",)
    if key not in _CACHE:
        _CACHE[key] = _build(use_bf16)
    return _CACHE[key]


def _make_in_maps(features, hidden_state, W_att, b_att, v):
    features = np.ascontiguousarray(features, dtype=np.float32)
    hidden_state = np.ascontiguousarray(hidden_state, dtype=np.float32)
    W_att = np.ascontiguousarray(W_att, dtype=np.float32)
    b_att = np.ascontiguousarray(b_att, dtype=np.float32).reshape(1, H)
    v = np.ascontiguousarray(v, dtype=np.float32).reshape(1, H)
    ident = np.eye(128, dtype=np.float32)
    ones = np.ones((128, 128), dtype=np.float32)
    in_maps = []
    for c in range(N_CORES):
        in_maps.append({
            "features": features[c * B_LOCAL:(c + 1) * B_LOCAL],
            "hidden_state": hidden_state[c * B_LOCAL:(c + 1) * B_LOCAL],
            "W_att": W_att,
            "b_att": b_att,
            "v": v,
            "ident": ident,
            "ones": ones,
        })
    return in_maps


def run_spmd(features, hidden_state, W_att, b_att, v, use_bf16=True, **kw):
    """Run on all 8 cores; returns BassKernelResults. kw passed through
    (e.g. trace=True, tmpdir=...)."""
    from concourse.bass_utils import run_bass_kernel_spmd
    nc = _get_nc(use_bf16)
    in_maps = _make_in_maps(features, hidden_state, W_att, b_att, v)
    return run_bass_kernel_spmd(nc, in_maps, list(range(N_CORES)), **kw)


def kernel(features, hidden_state, W_att, b_att, v):
    res = run_spmd(features, hidden_state, W_att, b_att, v)
    context = np.concatenate(
        [res.results[c]["context"] for c in range(N_CORES)], axis=0)
    attention_weights = np.concatenate(
        [res.results[c]["attention_weights"] for c in range(N_CORES)], axis=0)
    return context, attention_weights


# revision 12
# speedup vs baseline: 1.0457x; 1.0457x over previous
"""Trainium2 Bass kernel for additive (Bahdanau-style) attention.

reference:
    energy = tanh(features @ Wf + hidden @ Wh + b_att)   # [B, L, H]
    scores = energy @ v                                   # [B, L]
    attn   = softmax(scores, axis=1)                      # [B, L]
    context = einsum('blf,bl->bf', features, attn)        # [B, F]

Sharding: data-parallel over batch B=64 across 8 cores (8 batches/core),
small weights replicated.

Per-core design (per batch b, per l-group of 512 rows):
  - DMA features l-group natively [l=128, f] with fp32->bf16 cast (SWDGE)
  - PE-transpose (bf16) into [f, l] tiles; DVE evacuates PSUM->SBUF
  - main matmul projT[h, l] += Wf_chunk.T @ featT_chunk (bf16, fp32 PSUM)
  - ACT evacuates PSUM with fused tanh(proj + c[b][h]) (bias per-partition)
  - scores[l] = v . energy via thin PE matmul (accumulate over h-chunks)
  - softmax handled partition-major: exp on ACT, partition-sum via
    ones-matmul broadcast trick, normalize on DVE
  - context accumulated on PE with unnormalized exp weights, scaled by
    1/sum at the end of the batch
"""

import numpy as np

B_TOTAL = 64
B_LOCAL = 8
L = 2048
F = 512
H = 512
N_CORES = 8

_CACHE = {}


def _build(use_bf16=True):
    import concourse.bass as bass
    import concourse.tile as tile
    from concourse import bacc, mybir

    dt = mybir.dt
    feat_dt = dt.float16 if use_bf16 else dt.float32r
    AF = mybir.ActivationFunctionType

    LG = 512                 # l-group size
    n_lg = L // LG           # 4
    n_lt = LG // 128         # 4  (128-row subtiles per l-group)
    n_fc = F // 128          # 4  (feature chunks)
    n_hc = H // 128          # 4  (hidden chunks)
    n_col = n_lg * n_lt      # 16 (score columns per batch, partition-major)

    nc = bacc.Bacc("TRN2", target_bir_lowering=False, debug=False, num_devices=1)

    feats = nc.dram_tensor("features", [B_LOCAL, L, F], dt.float32, kind="ExternalInput").ap()
    hidden = nc.dram_tensor("hidden_state", [B_LOCAL, H], dt.float32, kind="ExternalInput").ap()
    w_att = nc.dram_tensor("W_att", [F + H, H], dt.float32, kind="ExternalInput").ap()
    b_att = nc.dram_tensor("b_att", [1, H], dt.float32, kind="ExternalInput").ap()
    v_in = nc.dram_tensor("v", [1, H], dt.float32, kind="ExternalInput").ap()
    ident_in = nc.dram_tensor("ident", [128, 128], dt.float32, kind="ExternalInput").ap()
    ones_in = nc.dram_tensor("ones", [128, 128], dt.float32, kind="ExternalInput").ap()

    ctx_out = nc.dram_tensor("context", [B_LOCAL, F], dt.float32, kind="ExternalOutput").ap()
    attn_out = nc.dram_tensor("attention_weights", [B_LOCAL, L], dt.float32, kind="ExternalOutput").ap()

    with tile.TileContext(nc) as tc:
        with tc.tile_pool(name="const", bufs=1) as const:
            identf = const.tile([128, 128], dt.float32, tag="identf")
            nc.sync.dma_start(identf[:], ident_in[:])
            ones_lp = const.tile([128, 128], feat_dt, tag="ones_lp")
            nc.gpsimd.dma_start(ones_lp[:], ones_in[:])  # cast f32->fp16
            if use_bf16:
                identb = const.tile([128, 128], feat_dt, tag="identb")
                nc.gpsimd.dma_start(identb[:], ident_in[:])  # cast f32->bf16
            else:
                identb = identf.bitcast(dt.float32r)
            # Wf chunks [f=128, H] in feat dtype
            wf = []
            for fc in range(n_fc):
                t = const.tile([128, H], feat_dt, tag=f"wf{fc}")
                if use_bf16:
                    nc.gpsimd.dma_start(t[:], w_att[fc * 128:(fc + 1) * 128, :])
                else:
                    nc.sync.dma_start(t.bitcast(dt.float32)[:], w_att[fc * 128:(fc + 1) * 128, :])
                wf.append(t)
            # persistent small tensors built in precompute below
            vt = const.tile([128, n_hc], feat_dt, tag="vt")          # v, h-chunk cols
            ct = const.tile([128, n_hc * B_LOCAL], dt.float32, tag="ct")  # c[b][h], per h-chunk

            # ---- precompute c = hidden @ Wh + b_att (fp32), vT ----
            with tc.tile_pool(name="pre", bufs=1) as pre, \
                 tc.tile_pool(name="prepsum", bufs=2, space="PSUM") as prepsum:
                hid = pre.tile([B_LOCAL, H], dt.float32, tag="hid")
                nc.sync.dma_start(hid[:], hidden[:])
                wh = []
                for fc in range(n_fc):
                    t = pre.tile([128, H], dt.float32, tag=f"wh{fc}")
                    nc.sync.dma_start(t[:], w_att[F + fc * 128:F + (fc + 1) * 128, :])
                    wh.append(t)
                brow = pre.tile([1, H], dt.float32, tag="brow")
                nc.sync.dma_start(brow[:], b_att[:])
                vrow = pre.tile([1, H], dt.float32, tag="vrow")
                nc.sync.dma_start(vrow[:], v_in[:])

                # hidden transposed: ht[fc] = [f=128, B_LOCAL]
                ht = []
                for fc in range(n_fc):
                    p = prepsum.tile([128, B_LOCAL], dt.float32, tag="pre_t")
                    nc.tensor.transpose(
                        p[:], hid[:, fc * 128:(fc + 1) * 128], identf[:B_LOCAL, :B_LOCAL])
                    t = pre.tile([128, B_LOCAL], dt.float32, tag=f"ht{fc}")
                    nc.vector.tensor_copy(t[:], p[:])
                    ht.append(t)
                # b_att transposed: battT [128, n_hc] (col hc = b_att[hc*128:...])
                pb = prepsum.tile([128, n_hc], dt.float32, tag="pre_t")
                for hc in range(n_hc):
                    nc.tensor.transpose(
                        pb[:, hc:hc + 1], brow[0:1, hc * 128:(hc + 1) * 128], identf[0:1, 0:1])
                battT = pre.tile([128, n_hc], dt.float32, tag="battT")
                nc.vector.tensor_copy(battT[:], pb[:])
                # v transposed -> vt (cast to feat dtype)
                pv = prepsum.tile([128, n_hc], dt.float32, tag="pre_t")
                for hc in range(n_hc):
                    nc.tensor.transpose(
                        pv[:, hc:hc + 1], vrow[0:1, hc * 128:(hc + 1) * 128], identf[0:1, 0:1])
                if use_bf16:
                    nc.vector.tensor_copy(vt[:], pv[:])
                else:
                    nc.vector.tensor_copy(vt.bitcast(dt.float32)[:], pv[:])
                # cT[h=128, b] per h-chunk: sum_f Wh[f, h] * hT[f, b] + b_att[h]
                for hc in range(n_hc):
                    p = prepsum.tile([128, B_LOCAL], dt.float32, tag="pre_t")
                    for fc in range(n_fc):
                        nc.tensor.matmul(
                            p[:], lhsT=wh[fc][:, hc * 128:(hc + 1) * 128], rhs=ht[fc][:],
                            start=(fc == 0), stop=(fc == n_fc - 1))
                    nc.vector.tensor_scalar_add(
                        ct[:, hc * B_LOCAL:(hc + 1) * B_LOCAL], p[:], battT[:, hc:hc + 1])

            # ---- main pipeline (software-pipelined over (b, lg) steps) ----
            # stage head(k):  DMA nf, PE transposes, main MMs, ACT tanh -> en
            # stage tailA(k-1): scores MMs (en ready long ago), DVE srow copy
            # stage tailB(k-2): spm transposes, ACT exp, DVE fp16 cast
            # stage tailC(k-3): ctx MMs (exp ready long ago)
            # epilogue(b): after ctx of last lg of b
            with tc.tile_pool(name="nf", bufs=6) as nf_pool, \
                 tc.tile_pool(name="ft", bufs=n_fc) as ft_pool, \
                 tc.tile_pool(name="en", bufs=2 * n_hc) as en_pool, \
                 tc.tile_pool(name="srow", bufs=3) as srow_pool, \
                 tc.tile_pool(name="bt", bufs=3) as bt_pool, \
                 tc.tile_pool(name="psT", bufs=2, space="PSUM") as psT_pool, \
                 tc.tile_pool(name="psP", bufs=3, space="PSUM") as psP_pool, \
                 tc.tile_pool(name="psS", bufs=2, space="PSUM") as psS_pool, \
                 tc.tile_pool(name="psC", bufs=1, space="PSUM") as psC_pool:

                steps = [(b, lg) for b in range(B_LOCAL) for lg in range(n_lg)]
                st = {}        # step index -> dict of live tiles
                bstate = {}    # batch -> dict (exp_pm, exp_bf, pctx)

                def head(k):
                    b, lg = steps[k]
                    if lg == 0:
                        bstate[b] = {
                            "exp_pm": bt_pool.tile([128, n_col], dt.float32, tag="exp_pm", name="exp_pm"),
                            "exp_bf": bt_pool.tile([128, n_col], feat_dt, tag="exp_bf", name="exp_bf"),
                        }
                    nf = nf_pool.tile([128, n_lt * F], feat_dt, tag="nf")
                    src = feats[b, lg * LG:(lg + 1) * LG, :].rearrange(
                        "(n p) f -> p n f", p=128)
                    nc.gpsimd.dma_start(nf[:].rearrange("p (n f) -> p n f", f=F), src)
                    ft2 = []
                    for fp in range(n_fc // 2):
                        pT = psT_pool.tile([128, 2 * LG], feat_dt, tag="pT")
                        for half in range(2):
                            fc = 2 * fp + half
                            for lt in range(n_lt):
                                nc.tensor.transpose(
                                    pT[:, half * LG + lt * 128: half * LG + (lt + 1) * 128],
                                    nf[:, lt * F + fc * 128: lt * F + (fc + 1) * 128],
                                    identb[:])
                        t = ft_pool.tile([128, 2 * LG], feat_dt, tag="ft")
                        nc.vector.tensor_copy(t[:], pT[:])
                        ft2.append(t)
                    ft = [ft2[fc // 2][:, (fc % 2) * LG:(fc % 2 + 1) * LG] for fc in range(n_fc)]
                    st[k] = {"nf": nf, "ft": ft}

                def head2(k):
                    b, lg = steps[k]
                    s = st[k]
                    ft = s.pop("ft")
                    en = []
                    for hc in range(n_hc):
                        pproj = psP_pool.tile([128, LG], dt.float32, tag="pproj")
                        for fc in range(n_fc):
                            nc.tensor.matmul(
                                pproj[:],
                                lhsT=wf[fc][:, hc * 128:(hc + 1) * 128],
                                rhs=ft[fc][:],
                                start=(fc == 0), stop=(fc == n_fc - 1))
                        e = en_pool.tile([128, LG], feat_dt, tag="en")
                        nc.scalar.activation(
                            e[:], pproj[:], AF.Tanh,
                            bias=ct[:, hc * B_LOCAL + b: hc * B_LOCAL + b + 1])
                        en.append(e)
                    s["en"] = en

                def tailA(k, kc=None):
                    # scores MMs for step k (col-group 0), interleaved with
                    # ctx MMs for step kc (col-group at partition 32) so the
                    # two M=1 streams run concurrently on the PE array.
                    s = st[k]
                    psc = psS_pool.tile([1, LG], dt.float32, tag="psSm")
                    if kc is not None:
                        bc, lgc = steps[kc]
                        sc = st[kc]
                        bsc = bstate[bc]
                        if lgc == 0:
                            bsc["pctx"] = psC_pool.tile([33, F], dt.float32, tag="pctx", name="pctx")
                        c0c = lgc * n_lt
                    for hc in range(n_hc):
                        nc.tensor.matmul(
                            psc[:], lhsT=vt[:, hc:hc + 1], rhs=s["en"][hc][:],
                            start=(hc == 0), stop=(hc == n_hc - 1),
                            tile_position=(0, 0))
                        if kc is not None:
                            lt = hc
                            nc.tensor.matmul(
                                bsc["pctx"][32:33, :],
                                lhsT=bsc["exp_bf"][:, c0c + lt:c0c + lt + 1],
                                rhs=sc["nf"][:, lt * F:(lt + 1) * F],
                                start=(lgc == 0 and lt == 0),
                                stop=(lgc == n_lg - 1 and lt == n_lt - 1),
                                tile_position=(0, 32))
                    srow = srow_pool.tile([1, LG], dt.float32, tag="srow")
                    nc.vector.tensor_copy(srow[:], psc[:])
                    s["srow"] = srow
                    del s["en"]
                    if kc is not None:
                        del st[kc]

                def tailB(k):
                    b, lg = steps[k]
                    s = st[k]
                    bs = bstate[b]
                    pspm = psS_pool.tile([128, n_lt], dt.float32, tag="psSm")
                    for lt in range(n_lt):
                        nc.tensor.transpose(
                            pspm[:, lt:lt + 1],
                            s["srow"][0:1, lt * 128:(lt + 1) * 128],
                            identf[0:1, 0:1])
                    c0 = lg * n_lt
                    nc.scalar.activation(
                        bs["exp_pm"][:, c0:c0 + n_lt], pspm[:], AF.Exp)
                    nc.vector.tensor_copy(
                        bs["exp_bf"][:, c0:c0 + n_lt], bs["exp_pm"][:, c0:c0 + n_lt])
                    del s["srow"]

                def tailC(k):
                    b, lg = steps[k]
                    s = st[k]
                    bs = bstate[b]
                    if lg == 0:
                        bs["pctx"] = psC_pool.tile([33, F], dt.float32, tag="pctx", name="pctx")
                    c0 = lg * n_lt
                    for lt in range(n_lt):
                        nc.tensor.matmul(
                            bs["pctx"][32:33, :],
                            lhsT=bs["exp_bf"][:, c0 + lt:c0 + lt + 1],
                            rhs=s["nf"][:, lt * F:(lt + 1) * F],
                            start=(lg == 0 and lt == 0),
                            stop=(lg == n_lg - 1 and lt == n_lt - 1),
                            tile_position=(0, 32))
                    del st[k]

                def epilogue(b):
                    bs = bstate.pop(b)
                    pgs = psS_pool.tile([128, n_col], dt.float32, tag="psSm")
                    nc.tensor.matmul(
                        pgs[:], lhsT=ones_lp[:], rhs=bs["exp_bf"][:],
                        start=True, stop=True)
                    ssum = bt_pool.tile([128, 1], dt.float32, tag="ssum")
                    nc.vector.reduce_sum(ssum[:], pgs[:], axis=mybir.AxisListType.X)
                    rsum = bt_pool.tile([128, 1], dt.float32, tag="rsum")
                    nc.vector.reciprocal(rsum[:], ssum[:])
                    attn_pm = bt_pool.tile([128, n_col], dt.float32, tag="attn_pm")
                    nc.vector.tensor_scalar_mul(attn_pm[:], bs["exp_pm"][:], rsum[:, 0:1])
                    pat = psT_pool.tile([n_col, 128], dt.float32, tag="pT")
                    nc.tensor.transpose(pat[:], attn_pm[:], identf[:])
                    attn_rm = bt_pool.tile([n_col, 128], dt.float32, tag="attn_rm")
                    nc.vector.tensor_copy(attn_rm[:], pat[:])
                    nc.sync.dma_start(
                        attn_out[b].rearrange("(a c) -> a c", a=n_col), attn_rm[:])
                    ctx_sb = bt_pool.tile([1, F], dt.float32, tag="ctx_sb")
                    nc.vector.tensor_scalar_mul(ctx_sb[:], bs["pctx"][32:33, :], rsum[0:1, 0:1])
                    nc.sync.dma_start(ctx_out[b:b + 1, :], ctx_sb[:])

                n_steps = len(steps)
                for k in range(n_steps + 3):
                    if k < n_steps:
                        head(k)
                    if k >= 1 and k - 1 < n_steps:
                        tailA(k - 1, kc=(k - 3 if k >= 3 else None))
                    elif k >= 3 and k - 3 < n_steps:
                        tailC(k - 3)
                    if k >= 3 and k - 3 < n_steps and steps[k - 3][1] == n_lg - 1:
                        epilogue(steps[k - 3][0])
                    if k < n_steps:
                        head2(k)
                    if k >= 2 and k - 2 < n_steps:
                        tailB(k - 2)

    nc.compile()
    return nc


def _get_nc(use_bf16=True):
    key = ("bf16" if use_bf16 else "f32r",)
    if key not in _CACHE:
        _CACHE[key] = _build(use_bf16)
    return _CACHE[key]


def _make_in_maps(features, hidden_state, W_att, b_att, v):
    features = np.ascontiguousarray(features, dtype=np.float32)
    hidden_state = np.ascontiguousarray(hidden_state, dtype=np.float32)
    W_att = np.ascontiguousarray(W_att, dtype=np.float32)
    b_att = np.ascontiguousarray(b_att, dtype=np.float32).reshape(1, H)
    v = np.ascontiguousarray(v, dtype=np.float32).reshape(1, H)
    ident = np.eye(128, dtype=np.float32)
    ones = np.ones((128, 128), dtype=np.float32)
    in_maps = []
    for c in range(N_CORES):
        in_maps.append({
            "features": features[c * B_LOCAL:(c + 1) * B_LOCAL],
            "hidden_state": hidden_state[c * B_LOCAL:(c + 1) * B_LOCAL],
            "W_att": W_att,
            "b_att": b_att,
            "v": v,
            "ident": ident,
            "ones": ones,
        })
    return in_maps


def run_spmd(features, hidden_state, W_att, b_att, v, use_bf16=True, **kw):
    """Run on all 8 cores; returns BassKernelResults. kw passed through
    (e.g. trace=True, tmpdir=...)."""
    from concourse.bass_utils import run_bass_kernel_spmd
    nc = _get_nc(use_bf16)
    in_maps = _make_in_maps(features, hidden_state, W_att, b_att, v)
    return run_bass_kernel_spmd(nc, in_maps, list(range(N_CORES)), **kw)


def kernel(features, hidden_state, W_att, b_att, v):
    res = run_spmd(features, hidden_state, W_att, b_att, v)
    context = np.concatenate(
        [res.results[c]["context"] for c in range(N_CORES)], axis=0)
    attention_weights = np.concatenate(
        [res.results[c]["attention_weights"] for c in range(N_CORES)], axis=0)
    return context, attention_weights
